# revision 34
# baseline (speedup 1.0000x reference)
"""HSTU attention (B=2, L=2048, D=1024, H=16) on 8 TRN2 NeuronCores.

Sharding: batch (2) x head-group (4 heads, 256 features) -> 8 cores.
Host sums the 4 partial W_o outputs per batch.

Design (vs the v1 baseline, 284us -> 195us in the TimelineSim cost model):
  - S^T tiles for the 2 heads of an ec merged into one 2-bank PSUM tile
    [128, 2x512]; one exp per (ec, jc, ic) with the hybrid-mask bias as a
    per-partition column shared by both heads, output bf16. Diagonal
    (jc//4 == ic) pairs split the exp at the diagonal subtile: columns
    below use the prompt-only (ab) bias, columns from the diagonal up use
    the valid-only (bl) bias, and only the single diagonal subtile gets a
    [128,2,128] 2D mask add on DVE.
  - AV in O-layout: out[i, d|rowsum] with e-slices stationary and bf16
    [V | ones] (free 65) moving; accumulated over jc into PSUM slots
    packed 7-per-bank (no bank straddle), one start=True per bank per ic
    (zero-region covers the rest), remainder start=False with
    skip_group_check. Software-pipelined one pair deep: AV of pair p-1 is
    emitted after the S matmuls of pair p so PE never blocks on ACT.
  - av staged to SBUF (bf16) right after the last AV of an ic; gating
    (reciprocal + scalar_tensor_tensor with the per-slot recip as the
    per-partition scalar) runs on DVE off the staged copy, overlapped
    with the next ic's attention. G^T via PE transposes (identity matmul;
    the DMA-transpose XBAR path is numerically broken in this stack),
    W_o row-sharded from G^T, outputs staged by DVE/ACT copies.
  - seq_lens-aware: only njc = ceil(max(seq_lens)/128) key tiles are
    computed (compile cache keyed on njc); per-core bias columns mask the
    remainder. f32r DRAM tensors are DMA'd straight into SBUF (no convert
    copies). Q/U projections of ic+1, W_o of ic-1, and the K/V blocks
    2..3 (for ic 0) are interleaved into the attention pair stream as
    paced PE filler; x/weight preloads are split in halves and ordered so
    the first K projection starts ~4us in.
"""

import sys

for _p in ("/opt/trn_rl_repo", "/root/.axon_site/_ro/trn_rl_repo"):
    if _p not in sys.path:
        sys.path.insert(0, _p)

import ml_dtypes
import numpy as np

import concourse.bass as bass  # noqa: F401
import concourse.mybir as mybir
import concourse.tile as tile
from concourse import bacc
from concourse.bass_utils import run_bass_kernel_spmd

F32 = mybir.dt.float32
F32R = mybir.dt.float32r
BF16 = mybir.dt.bfloat16
EXP = mybir.ActivationFunctionType.Exp
MULT = mybir.AluOpType.mult

B, L, D, H = 2, 2048, 1024, 16
DK = D // H          # 64
HPC = 4              # heads per core
E = HPC * DK         # 256 features per core
NDC = D // 128       # 8 contraction chunks for projections
NIC = L // 512       # 4 query blocks
NEG = -10000.0

_cache = {}


def _slot(k):
    """PSUM offset (f32 words) of av slot k: 7 slots of 65 per 2KB bank."""
    return (k // 7) * 512 + (k % 7) * 65


def _build(njc):
    nc = bacc.Bacc("TRN2", target_bir_lowering=False, debug=False)

    xt = nc.dram_tensor("xt", [D, L], F32R, kind="ExternalInput").ap()
    wq = nc.dram_tensor("wq", [D, E], F32R, kind="ExternalInput").ap()
    wk = nc.dram_tensor("wk", [D, E], F32R, kind="ExternalInput").ap()
    wv = nc.dram_tensor("wv", [D, E], F32R, kind="ExternalInput").ap()
    wu = nc.dram_tensor("wu", [D, E], F32R, kind="ExternalInput").ap()
    wo = nc.dram_tensor("wo", [E, D], BF16, kind="ExternalInput").ap()
    biasab = nc.dram_tensor("biasab", [128, njc], F32, kind="ExternalInput").ap()
    biasbl = nc.dram_tensor("biasbl", [128, njc], F32, kind="ExternalInput").ap()
    dmask = nc.dram_tensor("dmask", [njc, 128, 512], BF16, kind="ExternalInput").ap()
    ident = nc.dram_tensor("ident", [128, 128], BF16, kind="ExternalInput").ap()
    out = nc.dram_tensor("out", [L, D], F32, kind="ExternalOutput").ap()

    with tile.TileContext(nc) as tc:
        with tc.tile_pool(name="persist", bufs=1) as P:
            kt = [P.tile([128, L], F32R, tag=f"kt{i}", name=f"kt{i}") for i in range(2)]
            qt = [P.tile([128, L], F32R, tag=f"qt{i}", name=f"qt{i}") for i in range(2)]
            ut = P.tile([128, 16, E], BF16, tag="ut", name="ut")
            vt = [
                P.tile([128, njc, 2, 65], BF16, tag=f"vt{i}", name=f"vt{i}")
                for i in range(2)
            ]
            gt = {
                (ec, lc): P.tile(
                    [128, 128], BF16, tag=f"gt{ec}_{lc}", name=f"gt{ec}_{lc}"
                )
                for ec in range(2)
                for lc in range(16)
            }
            wqr = P.tile([128, NDC, E], F32R, tag="wqr", name="wqr")
            wkr = P.tile([128, NDC, E], F32R, tag="wkr", name="wkr")
            wvr = P.tile([128, NDC, E], F32R, tag="wvr", name="wvr")
            wur = P.tile([128, NDC, E], F32R, tag="wur", name="wur")
            wor = P.tile([128, 2, D], BF16, tag="wor", name="wor")
            bab = P.tile([128, njc], F32, tag="bab", name="bab")
            bbl = P.tile([128, njc], F32, tag="bbl", name="bbl")
            dmt = P.tile([128, njc, 512], BF16, tag="dmt", name="dmt")
            idt = P.tile([128, 128], BF16, tag="idt", name="idt")

            # ones columns of vt (slot 64 of each [V|1] group)
            for ec in range(2):
                nc.vector.memset(vt[ec][:, :, :, 64:65], 1.0)

            def preload_early():
                half = NDC // 2
                nc.sync.dma_start(
                    out=wkr[:, 0:half, :],
                    in_=wk[0 : half * 128, :].rearrange("(dc p) e -> p dc e", p=128),
                )

            def preload_early2():
                half = NDC // 2
                nc.sync.dma_start(
                    out=wkr[:, half:NDC, :],
                    in_=wk[half * 128 : D, :].rearrange(
                        "(dc p) e -> p dc e", p=128
                    ),
                )
                nc.sync.dma_start(out=bab, in_=biasab)
                nc.sync.dma_start(out=bbl, in_=biasbl)
                nd = min(4, njc)
                nc.sync.dma_start(
                    out=dmt[:, 0:nd], in_=dmask[0:nd].rearrange("j p i -> p j i")
                )
                nc.sync.dma_start(
                    out=wvr, in_=wv.rearrange("(dc p) e -> p dc e", p=128)
                )
                nc.sync.dma_start(out=idt, in_=ident)

            def preload_mid():
                nc.sync.dma_start(
                    out=wqr, in_=wq.rearrange("(dc p) e -> p dc e", p=128)
                )
                nc.sync.dma_start(
                    out=wur, in_=wu.rearrange("(dc p) e -> p dc e", p=128)
                )

            def preload_late():
                if njc > 4:
                    nc.sync.dma_start(
                        out=dmt[:, 4:njc],
                        in_=dmask[4:njc].rearrange("j p i -> p j i"),
                    )
                nc.sync.dma_start(
                    out=wor, in_=wo.rearrange("(ec p) d -> p ec d", p=128)
                )

            with tc.tile_pool(name="xa", bufs=4) as xap, \
                 tc.tile_pool(name="epool", bufs=6) as epool, \
                 tc.tile_pool(name="gpool", bufs=8) as gpool, \
                 tc.tile_pool(name="avs", bufs=3) as avsp, \
                 tc.tile_pool(name="rpool", bufs=4) as rpool, \
                 tc.tile_pool(name="ostg", bufs=4) as ostg, \
                 tc.tile_pool(name="ps_s", bufs=2, space="PSUM") as ps_s, \
                 tc.tile_pool(name="ps_av", bufs=1, space="PSUM") as ps_av, \
                 tc.tile_pool(name="ps_pw", bufs=1, space="PSUM") as ps_pw:

                def load_x(ic):
                    t = xap.tile([128, NDC, 512], F32R, tag="x", name="xa")
                    half = NDC // 2
                    for hh in range(2):
                        nc.sync.dma_start(
                            out=t[:, hh * half : (hh + 1) * half, :],
                            in_=xt[
                                hh * half * 128 : (hh + 1) * half * 128,
                                ic * 512 : (ic + 1) * 512,
                            ].rearrange("(dc p) i -> p dc i", p=128),
                        )
                    return t

                def emit_kq(xa_t, ic, ec, wsrc, dst, pool=None, ptag=None):
                    isl = slice(ic * 512, (ic + 1) * 512)
                    p = (pool or ps_pw).tile(
                        [128, 512], F32, tag=ptag or "pw", name="pkq"
                    )
                    for dc in range(NDC):
                        nc.tensor.matmul(
                            p,
                            wsrc[:, dc, ec * 128 : (ec + 1) * 128],
                            xa_t[:, dc, :],
                            start=(dc == 0),
                            stop=(dc == NDC - 1),
                        )
                    nc.vector.tensor_copy(dst[ec][:, isl], p)

                def emit_v(xa_t, ic, it, pool=None, ptag=None):
                    lc = 4 * ic + it
                    p = (pool or ps_pw).tile(
                        [128, 512], F32, tag=ptag or "pw", name="pv"
                    )
                    pv = p[:, 0:E]
                    for dc in range(NDC):
                        nc.tensor.matmul(
                            pv,
                            xa_t[:, dc, it * 128 : (it + 1) * 128],
                            wvr[:, dc, :],
                            start=(dc == 0),
                            stop=(dc == NDC - 1),
                        )
                    with nc.allow_low_precision(reason="bf16 V"):
                        for ec in range(2):
                            nc.vector.tensor_copy(
                                vt[ec][:, lc, :, 0:64],
                                pv[:, ec * 128 : (ec + 1) * 128].rearrange(
                                    "p (h v) -> p h v", h=2
                                ),
                            )

                def emit_u(xa_t, ic, it, pool=None, ptag=None):
                    lc = 4 * ic + it
                    p = (pool or ps_pw).tile(
                        [128, 512], F32, tag=ptag or "pw", name="pu"
                    )
                    pu = p[:, 0:E]
                    for dc in range(NDC):
                        nc.tensor.matmul(
                            pu,
                            xa_t[:, dc, it * 128 : (it + 1) * 128],
                            wur[:, dc, :],
                            start=(dc == 0),
                            stop=(dc == NDC - 1),
                        )
                    with nc.allow_low_precision(reason="bf16 U"):
                        nc.vector.tensor_copy(ut[:, lc, :], pu)

                ostate = {}

                def emit_wo(ic, it, fc, pool=None, ptag=None, copier=None):
                    lc = 4 * ic + it
                    if fc == 0:
                        ostate[lc] = ostg.tile([128, 1024], F32, tag="o", name="ostg")
                    p = (pool or ps_pw).tile(
                        [128, 512], F32, tag=ptag or "pw", name="pwo"
                    )
                    for ec in range(2):
                        nc.tensor.matmul(
                            p,
                            gt[(ec, lc)],
                            wor[:, ec, fc * 512 : (fc + 1) * 512],
                            start=(ec == 0),
                            stop=(ec == 1),
                        )
                    o = ostate[lc]
                    cp = copier or nc.vector
                    if cp is nc.scalar:
                        cp.copy(o[:, fc * 512 : (fc + 1) * 512], p)
                    else:
                        cp.tensor_copy(o[:, fc * 512 : (fc + 1) * 512], p)
                    if fc == 1:
                        nc.sync.dma_start(
                            out=out[lc * 128 : (lc + 1) * 128, :], in_=o
                        )

                def wo_filler(ic):
                    for it in range(4):
                        for fc in range(2):
                            yield 1024, (lambda ic=ic, it=it, fc=fc: emit_wo(ic, it, fc))

                def qu_filler(xa_t, ic, pool=None, ptag=None):
                    for ec in range(2):
                        yield 4096, (lambda ec=ec: emit_kq(
                            xa_t, ic, ec, wqr, qt, pool=pool, ptag=ptag
                        ))
                    for it in range(4):
                        yield 2048, (lambda it=it: emit_u(
                            xa_t, ic, it, pool=pool, ptag=ptag
                        ))

                # ---------- upfront: K/V for blocks 0-1; later blocks are
                # interleaved into ic=0's attention as filler work ----------
                nkb = -(-njc // 4)  # 512-blocks of keys needed
                half = NDC // 2
                xs = {}
                x0 = xap.tile([128, NDC, 512], F32R, tag="x", name="xa")
                xs[0] = x0
                nc.sync.dma_start(
                    out=x0[:, 0:half, :],
                    in_=xt[0 : half * 128, 0:512].rearrange(
                        "(dc p) i -> p dc i", p=128
                    ),
                )
                preload_early()
                nc.sync.dma_start(
                    out=x0[:, half:NDC, :],
                    in_=xt[half * 128 : D, 0:512].rearrange(
                        "(dc p) i -> p dc i", p=128
                    ),
                )
                preload_early2()
                nkb_up = min(nkb, 2)
                for ic in range(nkb_up):
                    if ic + 1 < NIC:
                        xs[ic + 1] = load_x(ic + 1)
                    if ic == 0:
                        preload_mid()
                    if ic == 1 or nkb_up == 1:
                        preload_late()
                    for ec in range(2):
                        emit_kq(xs[ic], ic, ec, wkr, kt, pool=ps_s, ptag="s")
                    for it in range(4):
                        if 4 * ic + it < njc:
                            emit_v(xs[ic], ic, it, pool=ps_s, ptag="s")
                for ic in range(nkb_up, NIC):
                    if ic + 1 < NIC:
                        xs[ic + 1] = load_x(ic + 1)

                def kv_filler(ic):
                    for ec in range(2):
                        yield 4096, (lambda ec=ec: emit_kq(xs[ic], ic, ec, wkr, kt))
                    for it in range(4):
                        if 4 * ic + it < njc:
                            yield 2048, (lambda it=it: emit_v(xs[ic], ic, it))

                # Q/U for ic=0 (emitted before attention)
                for _, f in qu_filler(xs[0], 0, pool=ps_s, ptag="s"):
                    f()

                def gate_ic(ic, avs, fuse_wo=False):
                    avsr = avs.rearrange("p (s c) -> p s c", c=65)
                    rec = rpool.tile([128, 16], F32, tag="rec", name="rec")
                    with nc.allow_low_precision(reason="softmax recip"):
                        nc.vector.reciprocal(rec, avsr[:, :, 64])
                    for it in range(4):
                        lc = 4 * ic + it
                        for ec in range(2):
                            g = gpool.tile([128, 128], BF16, tag="g", name="g")
                            with nc.allow_low_precision(reason="bf16 gating"):
                                for h in range(2):
                                    grp = 2 * ec + h
                                    k = it * 4 + grp
                                    nc.vector.scalar_tensor_tensor(
                                        g[:, h * 64 : (h + 1) * 64],
                                        avsr[:, k, 0:64],
                                        rec[:, k : k + 1],
                                        ut[:, lc, grp * 64 : (grp + 1) * 64],
                                        MULT,
                                        MULT,
                                    )
                            pt = (ps_s if fuse_wo else ps_pw).tile(
                                [128, 128], BF16,
                                tag="s" if fuse_wo else "pw", name="pt"
                            )
                            nc.tensor.transpose(pt, g, idt)
                            with nc.allow_low_precision(reason="bf16 gt"):
                                if fuse_wo:
                                    nc.scalar.copy(gt[(ec, lc)], pt)
                                else:
                                    nc.vector.tensor_copy(gt[(ec, lc)], pt)
                        if fuse_wo:
                            for fc in range(2):
                                emit_wo(
                                    ic, it, fc, pool=ps_s, ptag="s",
                                    copier=nc.scalar,
                                )

                # ---------- main loop ----------
                pend_gate = None
                for ic in range(NIC):
                    fillers = []
                    if ic == 0:
                        for b in range(nkb_up, nkb):
                            fillers.extend(kv_filler(b))
                    if ic > 0:
                        fillers.extend(wo_filler(ic - 1))
                    if ic + 1 < NIC:
                        fillers.extend(qu_filler(xs[ic + 1], ic + 1))
                    fillers.reverse()  # pop() from the front of the schedule

                    av = ps_av.tile([128, 1536], F32, tag="av", name="av")
                    av_banks = set(range(3))  # no start=True: memset below
                    nc.vector.memset(av, 0.0)
                    if pend_gate is not None:
                        gate_ic(*pend_gate)
                        pend_gate = None

                    isl = slice(ic * 512, (ic + 1) * 512)
                    wtot = sum(w for w, _ in fillers) or 1
                    npair = 2 * njc
                    pair = 0
                    wdone = 0
                    pend_av = []

                    def emit_av(e, jc, ec):
                        for h in range(2):
                            grp = 2 * ec + h
                            for it in range(4):
                                k = it * 4 + grp
                                bank = k // 7
                                st = bank not in av_banks
                                av_banks.add(bank)
                                nc.tensor.matmul(
                                    av[:, _slot(k) : _slot(k) + 65],
                                    e[:, h, it, :],
                                    vt[ec][:, jc, h, :],
                                    start=st,
                                    stop=False,
                                    skip_group_check=True,
                                )

                    for jc in range(njc):
                        jsl = slice(jc * 128, (jc + 1) * 128)
                        for ec in range(2):
                            stile = ps_s.tile([128, 1024], F32, tag="s", name="stile")
                            for h in range(2):
                                nc.tensor.matmul(
                                    stile[:, h * 512 : (h + 1) * 512],
                                    kt[ec][h * 64 : (h + 1) * 64, jsl],
                                    qt[ec][h * 64 : (h + 1) * 64, isl],
                                    start=True,
                                    stop=True,
                                )
                            # AV of an earlier pair runs while ACT does this
                            # pair's exp (2-deep software pipeline)
                            if len(pend_av) >= 4:
                                emit_av(*pend_av.pop(0))
                            # paced filler work (Q/U of ic+1, W_o of ic-1)
                            pair += 1
                            while fillers and wdone * npair < pair * wtot:
                                w, f = fillers.pop()
                                wdone += w
                                f()
                            e = epool.tile([128, 2, 4, 128], BF16, tag="e", name="e")
                            sv = stile.rearrange("p (h i) -> p h i", h=2)
                            if jc // 4 == ic:
                                # diagonal pair: columns below the diagonal
                                # subtile see the prompt-only (ab) bias; the
                                # diagonal subtile needs the 2D mask; columns
                                # above are causally free (bl bias)
                                t0 = jc % 4
                                w0, w1 = t0 * 128, (t0 + 1) * 128
                                nc.vector.tensor_add(
                                    sv[:, :, w0:w1],
                                    sv[:, :, w0:w1],
                                    dmt[:, jc, w0:w1].unsqueeze(1).broadcast_to(
                                        [128, 2, 128]
                                    ),
                                )
                                with nc.allow_low_precision(reason="bf16 softmax"):
                                    if t0 > 0:
                                        nc.scalar.activation(
                                            e[:, :, 0:t0, :],
                                            sv[:, :, 0:w0].rearrange(
                                                "p h (it i) -> p h it i", i=128
                                            ),
                                            EXP,
                                            bias=bab[:, jc : jc + 1],
                                            scale=1.0,
                                        )
                                    nc.scalar.activation(
                                        e[:, :, t0:4, :],
                                        sv[:, :, w0:512].rearrange(
                                            "p h (it i) -> p h it i", i=128
                                        ),
                                        EXP,
                                        bias=bbl[:, jc : jc + 1],
                                        scale=1.0,
                                    )
                            else:
                                bias = bab if jc // 4 > ic else bbl
                                with nc.allow_low_precision(reason="bf16 softmax"):
                                    nc.scalar.activation(
                                        e.rearrange("p h it i -> p (h it i)"),
                                        stile,
                                        EXP,
                                        bias=bias[:, jc : jc + 1],
                                        scale=1.0,
                                    )
                            pend_av.append((e, jc, ec))
                    for pa in pend_av:
                        emit_av(*pa)
                    while fillers:
                        fillers.pop()[1]()
                    # stage av to SBUF (bf16) so gating doesn't block next memset
                    avs = avsp.tile([128, 16 * 65], BF16, tag="avs", name="avs")
                    with nc.allow_low_precision(reason="bf16 av staging"):
                        for r in range(3):
                            cnt = min(7, 16 - 7 * r)
                            nc.vector.tensor_copy(
                                avs[:, 7 * r * 65 : (7 * r + cnt) * 65],
                                av[:, r * 512 : r * 512 + cnt * 65],
                            )
                    pend_gate = (ic, avs)

                gate_ic(*pend_gate, fuse_wo=True)

    nc.compile()
    return nc


def _host_inputs(njc, x, token_types, seq_lens, W_q, W_k, W_v, W_u, W_o):
    x = np.asarray(x, dtype=np.float32)
    token_types = np.asarray(token_types)
    seq_lens = np.asarray(seq_lens)
    W_q = np.asarray(W_q, dtype=np.float32)
    W_k = np.asarray(W_k, dtype=np.float32)
    W_v = np.asarray(W_v, dtype=np.float32)
    W_u = np.asarray(W_u, dtype=np.float32)
    W_o = np.asarray(W_o, dtype=np.float32)

    per_batch = []
    jr = np.arange(L)
    for b in range(B):
        xtb = np.ascontiguousarray(x[b].T)
        prompt = token_types[b] < 3
        valid = jr < int(seq_lens[b])
        ab = np.where(prompt & valid, 0.0, NEG).astype(np.float32)
        bl = np.where(valid, 0.0, NEG).astype(np.float32)
        biasab = np.ascontiguousarray(ab[: njc * 128].reshape(njc, 128).T)
        biasbl = np.ascontiguousarray(bl[: njc * 128].reshape(njc, 128).T)
        # diagonal-tile 2D masks, on top of the bl bias: NEG only where a
        # valid item key sits above the diagonal
        dmk = np.empty((njc, 128, 512), np.float32)
        for jc in range(njc):
            j = jr[jc * 128 : (jc + 1) * 128]
            i = np.arange((jc // 4) * 512, (jc // 4) * 512 + 512)
            allowed = valid[j][:, None] & (
                prompt[j][:, None] | (j[:, None] <= i[None, :])
            )
            full = np.where(allowed, 0.0, NEG).astype(np.float32)
            dmk[jc] = full - bl[j][:, None]
        per_batch.append((xtb, biasab, biasbl, dmk))

    in_maps = []
    for c in range(8):
        b, gi = c // 4, c % 4
        e0 = E * gi
        xtb, biasab, biasbl, dmk = per_batch[b]
        in_maps.append(
            {
                "xt": xtb,
                "wq": np.ascontiguousarray((W_q[e0 : e0 + E] / 8.0).T),
                "wk": np.ascontiguousarray(W_k[e0 : e0 + E].T),
                "wv": np.ascontiguousarray(W_v[e0 : e0 + E].T),
                "wu": np.ascontiguousarray(W_u[e0 : e0 + E].T),
                "wo": np.ascontiguousarray(W_o[:, e0 : e0 + E].T).astype(
                    ml_dtypes.bfloat16
                ),
                "ident": np.eye(128, dtype=np.float32).astype(ml_dtypes.bfloat16),
                "biasab": biasab,
                "biasbl": biasbl,
                "dmask": dmk.astype(ml_dtypes.bfloat16),
            }
        )
    return in_maps


def kernel(x, token_types, seq_lens, W_q, W_k, W_v, W_u, W_o, **_run_kwargs):
    seq_lens = np.asarray(seq_lens)
    njc = int(-(-int(seq_lens.max()) // 128))
    njc = max(1, min(njc, L // 128))
    if ("nc", njc) not in _cache:
        _cache[("nc", njc)] = _build(njc)
    nc = _cache[("nc", njc)]
    _cache["nc"] = nc
    in_maps = _host_inputs(njc, x, token_types, seq_lens, W_q, W_k, W_v, W_u, W_o)
    try:
        res = run_bass_kernel_spmd(nc, in_maps, list(range(8)), **_run_kwargs)
    except Exception as ex:  # transient NRT device wedge: retry once
        if "UNRECOVERABLE" not in str(ex) and "UNAVAILABLE" not in str(ex):
            raise
        res = run_bass_kernel_spmd(nc, in_maps, list(range(8)), **_run_kwargs)
    _cache["last_result"] = res
    full = np.zeros((B, L, D), np.float64)
    for c in range(8):
        full[c // 4] += res.results[c]["out"].astype(np.float64)
    return full.astype(np.float32)


# revision 43
# speedup vs baseline: 1.0102x; 1.0102x over previous
"""HSTU attention (B=2, L=2048, D=1024, H=16) on 8 TRN2 NeuronCores.

Sharding: batch (2) x head-group (4 heads, 256 features) -> 8 cores.
Host sums the 4 partial W_o outputs per batch.

Design (vs the v1 baseline, 284us -> 195us in the TimelineSim cost model):
  - S^T tiles for the 2 heads of an ec merged into one 2-bank PSUM tile
    [128, 2x512]; one exp per (ec, jc, ic) with the hybrid-mask bias as a
    per-partition column shared by both heads, output bf16. Diagonal
    (jc//4 == ic) pairs split the exp at the diagonal subtile: columns
    below use the prompt-only (ab) bias, columns from the diagonal up use
    the valid-only (bl) bias, and only the single diagonal subtile gets a
    [128,2,128] 2D mask add on DVE.
  - AV in O-layout: out[i, d|rowsum] with e-slices stationary and bf16
    [V | ones] (free 65) moving; accumulated over jc into PSUM slots
    packed 7-per-bank (no bank straddle), one start=True per bank per ic
    (zero-region covers the rest), remainder start=False with
    skip_group_check. Software-pipelined four pairs deep: AV of pair p-4 is
    emitted after the S matmuls of pair p so PE never blocks on ACT;
    Q/U/W_o filler work is paced into the pair stream by estimated cost.
  - av staged to SBUF (bf16) right after the last AV of an ic; gating
    (reciprocal + scalar_tensor_tensor with the per-slot recip as the
    per-partition scalar) runs on DVE off the staged copy, overlapped
    with the next ic's attention. G^T via PE transposes (identity matmul;
    the DMA-transpose XBAR path is numerically broken in this stack),
    W_o row-sharded from G^T, outputs staged by DVE/ACT copies.
  - seq_lens-aware: only njc = ceil(max(seq_lens)/128) key tiles are
    computed (compile cache keyed on njc); per-core bias columns mask the
    remainder. f32r DRAM tensors are DMA'd straight into SBUF (no convert
    copies). Q/U projections of ic+1, W_o of ic-1, and the K/V blocks
    2..3 (for ic 0) are interleaved into the attention pair stream as
    paced PE filler; x/weight preloads are split in halves and ordered so
    the first K projection starts ~4us in.
"""

import sys

for _p in ("/opt/trn_rl_repo", "/root/.axon_site/_ro/trn_rl_repo"):
    if _p not in sys.path:
        sys.path.insert(0, _p)

import ml_dtypes
import numpy as np

import concourse.bass as bass  # noqa: F401
import concourse.mybir as mybir
import concourse.tile as tile
from concourse import bacc
from concourse.bass_utils import run_bass_kernel_spmd

F32 = mybir.dt.float32
F32R = mybir.dt.float32r
BF16 = mybir.dt.bfloat16
EXP = mybir.ActivationFunctionType.Exp
MULT = mybir.AluOpType.mult

B, L, D, H = 2, 2048, 1024, 16
DK = D // H          # 64
HPC = 4              # heads per core
E = HPC * DK         # 256 features per core
NDC = D // 128       # 8 contraction chunks for projections
NIC = L // 512       # 4 query blocks
NEG = -10000.0

_cache = {}


def _slot(k):
    """PSUM offset (f32 words) of av slot k: 7 slots of 65 per 2KB bank."""
    return (k // 7) * 512 + (k % 7) * 65


def _build(njc):
    nc = bacc.Bacc("TRN2", target_bir_lowering=False, debug=False)

    xt = nc.dram_tensor("xt", [D, L], F32R, kind="ExternalInput").ap()
    wq = nc.dram_tensor("wq", [D, E], F32R, kind="ExternalInput").ap()
    wk = nc.dram_tensor("wk", [D, E], F32R, kind="ExternalInput").ap()
    wv = nc.dram_tensor("wv", [D, E], F32R, kind="ExternalInput").ap()
    wu = nc.dram_tensor("wu", [D, E], F32R, kind="ExternalInput").ap()
    wo = nc.dram_tensor("wo", [E, D], BF16, kind="ExternalInput").ap()
    biasab = nc.dram_tensor("biasab", [128, njc], F32, kind="ExternalInput").ap()
    biasbl = nc.dram_tensor("biasbl", [128, njc], F32, kind="ExternalInput").ap()
    dmask = nc.dram_tensor("dmask", [njc, 128, 128], BF16, kind="ExternalInput").ap()
    ident = nc.dram_tensor("ident", [128, 128], BF16, kind="ExternalInput").ap()
    out = nc.dram_tensor("out", [L, D], F32, kind="ExternalOutput").ap()

    with tile.TileContext(nc) as tc:
        with tc.tile_pool(name="persist", bufs=1) as P:
            kt = [P.tile([128, L], F32R, tag=f"kt{i}", name=f"kt{i}") for i in range(2)]
            qt = [P.tile([128, L], F32R, tag=f"qt{i}", name=f"qt{i}") for i in range(2)]
            ut = P.tile([128, 16, E], BF16, tag="ut", name="ut")
            vt = [
                P.tile([128, njc, 2, 65], BF16, tag=f"vt{i}", name=f"vt{i}")
                for i in range(2)
            ]
            gt = {
                (ec, lc): P.tile(
                    [128, 128], BF16, tag=f"gt{ec}_{lc}", name=f"gt{ec}_{lc}"
                )
                for ec in range(2)
                for lc in range(16)
            }
            wqr = P.tile([128, NDC, E], F32R, tag="wqr", name="wqr")
            wkr = P.tile([128, NDC, E], F32R, tag="wkr", name="wkr")
            wvr = P.tile([128, NDC, E], F32R, tag="wvr", name="wvr")
            wur = P.tile([128, NDC, E], F32R, tag="wur", name="wur")
            wor = P.tile([128, 2, D], BF16, tag="wor", name="wor")
            bab = P.tile([128, njc], F32, tag="bab", name="bab")
            bbl = P.tile([128, njc], F32, tag="bbl", name="bbl")
            dmt = P.tile([128, njc, 128], BF16, tag="dmt", name="dmt")
            idt = P.tile([128, 128], BF16, tag="idt", name="idt")

            # ones columns of vt (slot 64 of each [V|1] group)
            for ec in range(2):
                nc.vector.memset(vt[ec][:, :, :, 64:65], 1.0)

            def preload_early():
                half = NDC // 2
                nc.sync.dma_start(
                    out=wkr[:, 0:half, :],
                    in_=wk[0 : half * 128, :].rearrange("(dc p) e -> p dc e", p=128),
                )

            def preload_early2():
                half = NDC // 2
                nc.sync.dma_start(
                    out=wkr[:, half:NDC, :],
                    in_=wk[half * 128 : D, :].rearrange(
                        "(dc p) e -> p dc e", p=128
                    ),
                )
                nc.sync.dma_start(out=bab, in_=biasab)
                nc.sync.dma_start(out=bbl, in_=biasbl)
                nd = min(4, njc)
                nc.sync.dma_start(
                    out=dmt[:, 0:nd], in_=dmask[0:nd].rearrange("j p i -> p j i")
                )
                nc.sync.dma_start(
                    out=wvr, in_=wv.rearrange("(dc p) e -> p dc e", p=128)
                )
                nc.sync.dma_start(out=idt, in_=ident)

            def preload_mid():
                nc.sync.dma_start(
                    out=wqr, in_=wq.rearrange("(dc p) e -> p dc e", p=128)
                )
                nc.sync.dma_start(
                    out=wur, in_=wu.rearrange("(dc p) e -> p dc e", p=128)
                )

            def preload_late():
                if njc > 4:
                    nc.sync.dma_start(
                        out=dmt[:, 4:njc],
                        in_=dmask[4:njc].rearrange("j p i -> p j i"),
                    )
                nc.sync.dma_start(
                    out=wor, in_=wo.rearrange("(ec p) d -> p ec d", p=128)
                )

            with tc.tile_pool(name="xa", bufs=4) as xap, \
                 tc.tile_pool(name="epool", bufs=7) as epool, \
                 tc.tile_pool(name="gpool", bufs=8) as gpool, \
                 tc.tile_pool(name="avs", bufs=3) as avsp, \
                 tc.tile_pool(name="rpool", bufs=4) as rpool, \
                 tc.tile_pool(name="ostg", bufs=4) as ostg, \
                 tc.tile_pool(name="ps_s", bufs=2, space="PSUM") as ps_s, \
                 tc.tile_pool(name="ps_av", bufs=1, space="PSUM") as ps_av, \
                 tc.tile_pool(name="ps_pw", bufs=1, space="PSUM") as ps_pw:

                def load_x(ic):
                    t = xap.tile([128, NDC, 512], F32R, tag="x", name="xa")
                    half = NDC // 2
                    for hh in range(2):
                        nc.sync.dma_start(
                            out=t[:, hh * half : (hh + 1) * half, :],
                            in_=xt[
                                hh * half * 128 : (hh + 1) * half * 128,
                                ic * 512 : (ic + 1) * 512,
                            ].rearrange("(dc p) i -> p dc i", p=128),
                        )
                    return t

                def emit_kq(xa_t, ic, ec, wsrc, dst, pool=None, ptag=None):
                    isl = slice(ic * 512, (ic + 1) * 512)
                    p = (pool or ps_pw).tile(
                        [128, 512], F32, tag=ptag or "pw", name="pkq"
                    )
                    for dc in range(NDC):
                        nc.tensor.matmul(
                            p,
                            wsrc[:, dc, ec * 128 : (ec + 1) * 128],
                            xa_t[:, dc, :],
                            start=(dc == 0),
                            stop=(dc == NDC - 1),
                        )
                    nc.vector.tensor_copy(dst[ec][:, isl], p)

                def emit_v(xa_t, ic, it, pool=None, ptag=None):
                    lc = 4 * ic + it
                    p = (pool or ps_pw).tile(
                        [128, 512], F32, tag=ptag or "pw", name="pv"
                    )
                    pv = p[:, 0:E]
                    for dc in range(NDC):
                        nc.tensor.matmul(
                            pv,
                            xa_t[:, dc, it * 128 : (it + 1) * 128],
                            wvr[:, dc, :],
                            start=(dc == 0),
                            stop=(dc == NDC - 1),
                        )
                    with nc.allow_low_precision(reason="bf16 V"):
                        for ec in range(2):
                            nc.vector.tensor_copy(
                                vt[ec][:, lc, :, 0:64],
                                pv[:, ec * 128 : (ec + 1) * 128].rearrange(
                                    "p (h v) -> p h v", h=2
                                ),
                            )

                def emit_u(xa_t, ic, it, pool=None, ptag=None):
                    lc = 4 * ic + it
                    p = (pool or ps_pw).tile(
                        [128, 512], F32, tag=ptag or "pw", name="pu"
                    )
                    pu = p[:, 0:E]
                    for dc in range(NDC):
                        nc.tensor.matmul(
                            pu,
                            xa_t[:, dc, it * 128 : (it + 1) * 128],
                            wur[:, dc, :],
                            start=(dc == 0),
                            stop=(dc == NDC - 1),
                        )
                    with nc.allow_low_precision(reason="bf16 U"):
                        nc.vector.tensor_copy(ut[:, lc, :], pu)

                ostate = {}

                def emit_wo(ic, it, fc, pool=None, ptag=None, copier=None):
                    lc = 4 * ic + it
                    if fc == 0:
                        ostate[lc] = ostg.tile([128, 1024], F32, tag="o", name="ostg")
                    p = (pool or ps_pw).tile(
                        [128, 512], F32, tag=ptag or "pw", name="pwo"
                    )
                    for ec in range(2):
                        nc.tensor.matmul(
                            p,
                            gt[(ec, lc)],
                            wor[:, ec, fc * 512 : (fc + 1) * 512],
                            start=(ec == 0),
                            stop=(ec == 1),
                        )
                    o = ostate[lc]
                    cp = copier or nc.vector
                    if cp is nc.scalar:
                        cp.copy(o[:, fc * 512 : (fc + 1) * 512], p)
                    else:
                        cp.tensor_copy(o[:, fc * 512 : (fc + 1) * 512], p)
                    if fc == 1:
                        nc.sync.dma_start(
                            out=out[lc * 128 : (lc + 1) * 128, :], in_=o
                        )

                def wo_filler(ic):
                    for it in range(4):
                        for fc in range(2):
                            yield 1024, (lambda ic=ic, it=it, fc=fc: emit_wo(ic, it, fc))

                def qu_filler(xa_t, ic, pool=None, ptag=None):
                    for ec in range(2):
                        yield 4096, (lambda ec=ec: emit_kq(
                            xa_t, ic, ec, wqr, qt, pool=pool, ptag=ptag
                        ))
                    for it in range(4):
                        yield 2048, (lambda it=it: emit_u(
                            xa_t, ic, it, pool=pool, ptag=ptag
                        ))

                # ---------- upfront: K/V for blocks 0-1; later blocks are
                # interleaved into ic=0's attention as filler work ----------
                nkb = -(-njc // 4)  # 512-blocks of keys needed
                half = NDC // 2
                xs = {}
                x0 = xap.tile([128, NDC, 512], F32R, tag="x", name="xa")
                xs[0] = x0
                nc.sync.dma_start(
                    out=x0[:, 0:half, :],
                    in_=xt[0 : half * 128, 0:512].rearrange(
                        "(dc p) i -> p dc i", p=128
                    ),
                )
                preload_early()
                nc.sync.dma_start(
                    out=x0[:, half:NDC, :],
                    in_=xt[half * 128 : D, 0:512].rearrange(
                        "(dc p) i -> p dc i", p=128
                    ),
                )
                preload_early2()
                nkb_up = min(nkb, 2)
                for ic in range(nkb_up):
                    if ic + 1 < NIC:
                        xs[ic + 1] = load_x(ic + 1)
                    if ic == 0:
                        preload_mid()
                    if ic == 1 or nkb_up == 1:
                        preload_late()
                    for ec in range(2):
                        emit_kq(xs[ic], ic, ec, wkr, kt, pool=ps_s, ptag="s")
                    for it in range(4):
                        if 4 * ic + it < njc:
                            emit_v(xs[ic], ic, it, pool=ps_s, ptag="s")
                for ic in range(nkb_up, NIC):
                    if ic + 1 < NIC:
                        xs[ic + 1] = load_x(ic + 1)

                def kv_filler(ic):
                    for ec in range(2):
                        yield 4096, (lambda ec=ec: emit_kq(xs[ic], ic, ec, wkr, kt))
                    for it in range(4):
                        if 4 * ic + it < njc:
                            yield 2048, (lambda it=it: emit_v(xs[ic], ic, it))

                # Q/U for ic=0 (emitted before attention)
                for _, f in qu_filler(xs[0], 0, pool=ps_s, ptag="s"):
                    f()

                def gate_ic(ic, avs, fuse_wo=False):
                    avsr = avs.rearrange("p (s c) -> p s c", c=65)
                    rec = rpool.tile([128, 16], F32, tag="rec", name="rec")
                    with nc.allow_low_precision(reason="softmax recip"):
                        nc.vector.reciprocal(rec, avsr[:, :, 64])
                    for it in range(4):
                        lc = 4 * ic + it
                        for ec in range(2):
                            g = gpool.tile([128, 128], BF16, tag="g", name="g")
                            with nc.allow_low_precision(reason="bf16 gating"):
                                for h in range(2):
                                    grp = 2 * ec + h
                                    k = it * 4 + grp
                                    nc.vector.scalar_tensor_tensor(
                                        g[:, h * 64 : (h + 1) * 64],
                                        avsr[:, k, 0:64],
                                        rec[:, k : k + 1],
                                        ut[:, lc, grp * 64 : (grp + 1) * 64],
                                        MULT,
                                        MULT,
                                    )
                            pt = (ps_s if fuse_wo else ps_pw).tile(
                                [128, 128], BF16,
                                tag="s" if fuse_wo else "pw", name="pt"
                            )
                            nc.tensor.transpose(pt, g, idt)
                            with nc.allow_low_precision(reason="bf16 gt"):
                                if fuse_wo:
                                    nc.scalar.copy(gt[(ec, lc)], pt)
                                else:
                                    nc.vector.tensor_copy(gt[(ec, lc)], pt)
                        if fuse_wo:
                            for fc in range(2):
                                emit_wo(
                                    ic, it, fc, pool=ps_s, ptag="s",
                                    copier=nc.scalar,
                                )

                # ---------- main loop ----------
                pend_gate = None
                for ic in range(NIC):
                    fillers = []
                    if ic == 0:
                        for b in range(nkb_up, nkb):
                            fillers.extend(kv_filler(b))
                    if ic > 0:
                        fillers.extend(wo_filler(ic - 1))
                    if ic + 1 < NIC:
                        fillers.extend(qu_filler(xs[ic + 1], ic + 1))
                    fillers.reverse()  # pop() from the front of the schedule

                    av = ps_av.tile([128, 1536], F32, tag="av", name="av")
                    av_banks = set(range(3))  # no start=True: memset below
                    nc.vector.memset(av, 0.0)
                    if pend_gate is not None:
                        gate_ic(*pend_gate)
                        pend_gate = None

                    isl = slice(ic * 512, (ic + 1) * 512)
                    wtot = sum(w for w, _ in fillers) or 1
                    npair = 2 * njc
                    pair = 0
                    wdone = 0
                    pend_av = []

                    def emit_av(e, jc, ec):
                        for h in range(2):
                            grp = 2 * ec + h
                            for it in range(4):
                                k = it * 4 + grp
                                bank = k // 7
                                st = bank not in av_banks
                                av_banks.add(bank)
                                nc.tensor.matmul(
                                    av[:, _slot(k) : _slot(k) + 65],
                                    e[:, h, it, :],
                                    vt[ec][:, jc, h, :],
                                    start=st,
                                    stop=False,
                                    skip_group_check=True,
                                )

                    for jc in range(njc):
                        jsl = slice(jc * 128, (jc + 1) * 128)
                        for ec in range(2):
                            stile = ps_s.tile([128, 1024], F32, tag="s", name="stile")
                            for h in range(2):
                                nc.tensor.matmul(
                                    stile[:, h * 512 : (h + 1) * 512],
                                    kt[ec][h * 64 : (h + 1) * 64, jsl],
                                    qt[ec][h * 64 : (h + 1) * 64, isl],
                                    start=True,
                                    stop=True,
                                )
                            # AV of an earlier pair runs while ACT does this
                            # pair's exp (2-deep software pipeline)
                            if len(pend_av) >= 6:
                                emit_av(*pend_av.pop(0))
                            # paced filler work (Q/U of ic+1, W_o of ic-1)
                            pair += 1
                            while fillers and wdone * npair < pair * wtot:
                                w, f = fillers.pop()
                                wdone += w
                                f()
                            e = epool.tile([128, 2, 4, 128], BF16, tag="e", name="e")
                            sv = stile.rearrange("p (h i) -> p h i", h=2)
                            if jc // 4 == ic:
                                # diagonal pair: columns below the diagonal
                                # subtile see the prompt-only (ab) bias; the
                                # diagonal subtile needs the 2D mask; columns
                                # above are causally free (bl bias)
                                t0 = jc % 4
                                w0, w1 = t0 * 128, (t0 + 1) * 128
                                nc.vector.tensor_add(
                                    sv[:, :, w0:w1],
                                    sv[:, :, w0:w1],
                                    dmt[:, jc, :].unsqueeze(1).broadcast_to(
                                        [128, 2, 128]
                                    ),
                                )
                                with nc.allow_low_precision(reason="bf16 softmax"):
                                    if t0 > 0:
                                        nc.scalar.activation(
                                            e[:, :, 0:t0, :],
                                            sv[:, :, 0:w0].rearrange(
                                                "p h (it i) -> p h it i", i=128
                                            ),
                                            EXP,
                                            bias=bab[:, jc : jc + 1],
                                            scale=1.0,
                                        )
                                    nc.scalar.activation(
                                        e[:, :, t0:4, :],
                                        sv[:, :, w0:512].rearrange(
                                            "p h (it i) -> p h it i", i=128
                                        ),
                                        EXP,
                                        bias=bbl[:, jc : jc + 1],
                                        scale=1.0,
                                    )
                            else:
                                bias = bab if jc // 4 > ic else bbl
                                with nc.allow_low_precision(reason="bf16 softmax"):
                                    nc.scalar.activation(
                                        e.rearrange("p h it i -> p (h it i)"),
                                        stile,
                                        EXP,
                                        bias=bias[:, jc : jc + 1],
                                        scale=1.0,
                                    )
                            pend_av.append((e, jc, ec))
                    for pa in pend_av:
                        emit_av(*pa)
                    while fillers:
                        fillers.pop()[1]()
                    # stage av to SBUF (bf16) so gating doesn't block next memset
                    avs = avsp.tile([128, 16 * 65], BF16, tag="avs", name="avs")
                    with nc.allow_low_precision(reason="bf16 av staging"):
                        for r in range(3):
                            cnt = min(7, 16 - 7 * r)
                            nc.vector.tensor_copy(
                                avs[:, 7 * r * 65 : (7 * r + cnt) * 65],
                                av[:, r * 512 : r * 512 + cnt * 65],
                            )
                    pend_gate = (ic, avs)

                gate_ic(*pend_gate, fuse_wo=True)

    nc.compile()
    return nc


def _host_inputs(njc, x, token_types, seq_lens, W_q, W_k, W_v, W_u, W_o):
    x = np.asarray(x, dtype=np.float32)
    token_types = np.asarray(token_types)
    seq_lens = np.asarray(seq_lens)
    W_q = np.asarray(W_q, dtype=np.float32)
    W_k = np.asarray(W_k, dtype=np.float32)
    W_v = np.asarray(W_v, dtype=np.float32)
    W_u = np.asarray(W_u, dtype=np.float32)
    W_o = np.asarray(W_o, dtype=np.float32)

    per_batch = []
    jr = np.arange(L)
    for b in range(B):
        xtb = np.ascontiguousarray(x[b].T)
        prompt = token_types[b] < 3
        valid = jr < int(seq_lens[b])
        ab = np.where(prompt & valid, 0.0, NEG).astype(np.float32)
        bl = np.where(valid, 0.0, NEG).astype(np.float32)
        biasab = np.ascontiguousarray(ab[: njc * 128].reshape(njc, 128).T)
        biasbl = np.ascontiguousarray(bl[: njc * 128].reshape(njc, 128).T)
        # diagonal-tile 2D masks, on top of the bl bias: NEG only where a
        # valid item key sits above the diagonal
        dmk = np.empty((njc, 128, 128), np.float32)
        for jc in range(njc):
            j = jr[jc * 128 : (jc + 1) * 128]
            i = np.arange(jc * 128, (jc + 1) * 128)  # diagonal subtile only
            allowed = valid[j][:, None] & (
                prompt[j][:, None] | (j[:, None] <= i[None, :])
            )
            full = np.where(allowed, 0.0, NEG).astype(np.float32)
            dmk[jc] = full - bl[j][:, None]
        per_batch.append((xtb, biasab, biasbl, dmk))

    in_maps = []
    for c in range(8):
        b, gi = c // 4, c % 4
        e0 = E * gi
        xtb, biasab, biasbl, dmk = per_batch[b]
        in_maps.append(
            {
                "xt": xtb,
                "wq": np.ascontiguousarray((W_q[e0 : e0 + E] / 8.0).T),
                "wk": np.ascontiguousarray(W_k[e0 : e0 + E].T),
                "wv": np.ascontiguousarray(W_v[e0 : e0 + E].T),
                "wu": np.ascontiguousarray(W_u[e0 : e0 + E].T),
                "wo": np.ascontiguousarray(W_o[:, e0 : e0 + E].T).astype(
                    ml_dtypes.bfloat16
                ),
                "ident": np.eye(128, dtype=np.float32).astype(ml_dtypes.bfloat16),
                "biasab": biasab,
                "biasbl": biasbl,
                "dmask": dmk.astype(ml_dtypes.bfloat16),
            }
        )
    return in_maps


def kernel(x, token_types, seq_lens, W_q, W_k, W_v, W_u, W_o, **_run_kwargs):
    seq_lens = np.asarray(seq_lens)
    njc = int(-(-int(seq_lens.max()) // 128))
    njc = max(1, min(njc, L // 128))
    if ("nc", njc) not in _cache:
        _cache[("nc", njc)] = _build(njc)
    nc = _cache[("nc", njc)]
    _cache["nc"] = nc
    in_maps = _host_inputs(njc, x, token_types, seq_lens, W_q, W_k, W_v, W_u, W_o)
    try:
        res = run_bass_kernel_spmd(nc, in_maps, list(range(8)), **_run_kwargs)
    except Exception as ex:  # transient NRT device wedge: retry once
        if "UNRECOVERABLE" not in str(ex) and "UNAVAILABLE" not in str(ex):
            raise
        res = run_bass_kernel_spmd(nc, in_maps, list(range(8)), **_run_kwargs)
    _cache["last_result"] = res
    full = np.zeros((B, L, D), np.float64)
    for c in range(8):
        full[c // 4] += res.results[c]["out"].astype(np.float64)
    return full.astype(np.float32)


# revision 46
# speedup vs baseline: 1.0142x; 1.0040x over previous
"""HSTU attention (B=2, L=2048, D=1024, H=16) on 8 TRN2 NeuronCores.

Sharding: batch (2) x head-group (4 heads, 256 features) -> 8 cores.
Host sums the 4 partial W_o outputs per batch.

Design (vs the v1 baseline, 284us -> 183.6us in the TimelineSim cost model):
  - S^T tiles for the 2 heads of an ec merged into one 2-bank PSUM tile
    [128, 2x512]; one exp per (ec, jc, ic) with the hybrid-mask bias as a
    per-partition column shared by both heads, output bf16. Diagonal
    (jc//4 == ic) pairs split the exp at the diagonal subtile: columns
    below use the prompt-only (ab) bias, columns from the diagonal up use
    the valid-only (bl) bias, and only the single diagonal subtile gets a
    [128,2,128] 2D mask add on DVE.
  - AV in O-layout: out[i, d|rowsum] with e-slices stationary and bf16
    [V | ones] (free 65) moving; accumulated over jc into PSUM slots
    packed 7-per-bank (no bank straddle), one start=True per bank per ic
    (zero-region covers the rest), remainder start=False with
    skip_group_check. Software-pipelined six pairs deep: AV of pair p-6 is
    emitted after the S matmuls of pair p so PE never blocks on ACT;
    Q/U/W_o filler work is paced into the pair stream by estimated cost.
  - av staged to SBUF (bf16) right after the last AV of an ic; gating
    (reciprocal + scalar_tensor_tensor with the per-slot recip as the
    per-partition scalar) runs on DVE off the staged copy, overlapped
    with the next ic's attention. G^T via PE transposes (identity matmul;
    the DMA-transpose XBAR path is numerically broken in this stack),
    W_o row-sharded from G^T, outputs staged by DVE/ACT copies.
  - seq_lens-aware: only njc = ceil(max(seq_lens)/128) key tiles are
    computed (compile cache keyed on njc); per-core bias columns mask the
    remainder. f32r DRAM tensors are DMA'd straight into SBUF (no convert
    copies). Q/U projections of ic+1, W_o of ic-1, and the K/V blocks
    2..3 (for ic 0) are interleaved into the attention pair stream as
    paced PE filler; x/weight preloads are split in halves and ordered so
    the first K projection starts ~4us in.
"""

import sys

for _p in ("/opt/trn_rl_repo", "/root/.axon_site/_ro/trn_rl_repo"):
    if _p not in sys.path:
        sys.path.insert(0, _p)

import ml_dtypes
import numpy as np

import concourse.bass as bass  # noqa: F401
import concourse.mybir as mybir
import concourse.tile as tile
from concourse import bacc
from concourse.bass_utils import run_bass_kernel_spmd

F32 = mybir.dt.float32
F32R = mybir.dt.float32r
BF16 = mybir.dt.bfloat16
EXP = mybir.ActivationFunctionType.Exp
MULT = mybir.AluOpType.mult

B, L, D, H = 2, 2048, 1024, 16
DK = D // H          # 64
HPC = 4              # heads per core
E = HPC * DK         # 256 features per core
NDC = D // 128       # 8 contraction chunks for projections
NIC = L // 512       # 4 query blocks
NEG = -10000.0

_cache = {}


def _slot(k):
    """PSUM offset (f32 words) of av slot k: 7 slots of 65 per 2KB bank."""
    return (k // 7) * 512 + (k % 7) * 65


def _build(njc):
    nc = bacc.Bacc("TRN2", target_bir_lowering=False, debug=False)

    xt = nc.dram_tensor("xt", [D, L], F32R, kind="ExternalInput").ap()
    wq = nc.dram_tensor("wq", [D, E], F32R, kind="ExternalInput").ap()
    wk = nc.dram_tensor("wk", [D, E], F32R, kind="ExternalInput").ap()
    wv = nc.dram_tensor("wv", [D, E], F32R, kind="ExternalInput").ap()
    wu = nc.dram_tensor("wu", [D, E], F32R, kind="ExternalInput").ap()
    wo = nc.dram_tensor("wo", [E, D], BF16, kind="ExternalInput").ap()
    biasab = nc.dram_tensor("biasab", [128, njc], F32, kind="ExternalInput").ap()
    biasbl = nc.dram_tensor("biasbl", [128, njc], F32, kind="ExternalInput").ap()
    dmask = nc.dram_tensor("dmask", [njc, 128, 128], BF16, kind="ExternalInput").ap()
    ident = nc.dram_tensor("ident", [128, 128], BF16, kind="ExternalInput").ap()
    out = nc.dram_tensor("out", [L, D], BF16, kind="ExternalOutput").ap()

    with tile.TileContext(nc) as tc:
        with tc.tile_pool(name="persist", bufs=1) as P:
            kt = [P.tile([128, L], F32R, tag=f"kt{i}", name=f"kt{i}") for i in range(2)]
            qt = [P.tile([128, L], F32R, tag=f"qt{i}", name=f"qt{i}") for i in range(2)]
            ut = P.tile([128, 16, E], BF16, tag="ut", name="ut")
            vt = [
                P.tile([128, njc, 2, 65], BF16, tag=f"vt{i}", name=f"vt{i}")
                for i in range(2)
            ]
            gt = {
                (ec, lc): P.tile(
                    [128, 128], BF16, tag=f"gt{ec}_{lc}", name=f"gt{ec}_{lc}"
                )
                for ec in range(2)
                for lc in range(16)
            }
            wqr = P.tile([128, NDC, E], F32R, tag="wqr", name="wqr")
            wkr = P.tile([128, NDC, E], F32R, tag="wkr", name="wkr")
            wvr = P.tile([128, NDC, E], F32R, tag="wvr", name="wvr")
            wur = P.tile([128, NDC, E], F32R, tag="wur", name="wur")
            wor = P.tile([128, 2, D], BF16, tag="wor", name="wor")
            bab = P.tile([128, njc], F32, tag="bab", name="bab")
            bbl = P.tile([128, njc], F32, tag="bbl", name="bbl")
            dmt = P.tile([128, njc, 128], BF16, tag="dmt", name="dmt")
            idt = P.tile([128, 128], BF16, tag="idt", name="idt")

            # ones columns of vt (slot 64 of each [V|1] group)
            for ec in range(2):
                nc.vector.memset(vt[ec][:, :, :, 64:65], 1.0)

            def preload_early():
                half = NDC // 2
                nc.sync.dma_start(
                    out=wkr[:, 0:half, :],
                    in_=wk[0 : half * 128, :].rearrange("(dc p) e -> p dc e", p=128),
                )

            def preload_early2():
                half = NDC // 2
                nc.sync.dma_start(
                    out=wkr[:, half:NDC, :],
                    in_=wk[half * 128 : D, :].rearrange(
                        "(dc p) e -> p dc e", p=128
                    ),
                )
                nc.sync.dma_start(out=bab, in_=biasab)
                nc.sync.dma_start(out=bbl, in_=biasbl)
                nd = min(4, njc)
                nc.sync.dma_start(
                    out=dmt[:, 0:nd], in_=dmask[0:nd].rearrange("j p i -> p j i")
                )
                nc.sync.dma_start(
                    out=wvr, in_=wv.rearrange("(dc p) e -> p dc e", p=128)
                )
                nc.sync.dma_start(out=idt, in_=ident)

            def preload_mid():
                nc.sync.dma_start(
                    out=wqr, in_=wq.rearrange("(dc p) e -> p dc e", p=128)
                )
                nc.sync.dma_start(
                    out=wur, in_=wu.rearrange("(dc p) e -> p dc e", p=128)
                )

            def preload_late():
                if njc > 4:
                    nc.sync.dma_start(
                        out=dmt[:, 4:njc],
                        in_=dmask[4:njc].rearrange("j p i -> p j i"),
                    )
                nc.sync.dma_start(
                    out=wor, in_=wo.rearrange("(ec p) d -> p ec d", p=128)
                )

            with tc.tile_pool(name="xa", bufs=4) as xap, \
                 tc.tile_pool(name="epool", bufs=7) as epool, \
                 tc.tile_pool(name="gpool", bufs=8) as gpool, \
                 tc.tile_pool(name="avs", bufs=3) as avsp, \
                 tc.tile_pool(name="rpool", bufs=4) as rpool, \
                 tc.tile_pool(name="ostg", bufs=4) as ostg, \
                 tc.tile_pool(name="ps_s", bufs=2, space="PSUM") as ps_s, \
                 tc.tile_pool(name="ps_av", bufs=1, space="PSUM") as ps_av, \
                 tc.tile_pool(name="ps_pw", bufs=1, space="PSUM") as ps_pw:

                def load_x(ic):
                    t = xap.tile([128, NDC, 512], F32R, tag="x", name="xa")
                    half = NDC // 2
                    for hh in range(2):
                        nc.sync.dma_start(
                            out=t[:, hh * half : (hh + 1) * half, :],
                            in_=xt[
                                hh * half * 128 : (hh + 1) * half * 128,
                                ic * 512 : (ic + 1) * 512,
                            ].rearrange("(dc p) i -> p dc i", p=128),
                        )
                    return t

                def emit_kq(xa_t, ic, ec, wsrc, dst, pool=None, ptag=None):
                    isl = slice(ic * 512, (ic + 1) * 512)
                    p = (pool or ps_pw).tile(
                        [128, 512], F32, tag=ptag or "pw", name="pkq"
                    )
                    for dc in range(NDC):
                        nc.tensor.matmul(
                            p,
                            wsrc[:, dc, ec * 128 : (ec + 1) * 128],
                            xa_t[:, dc, :],
                            start=(dc == 0),
                            stop=(dc == NDC - 1),
                        )
                    nc.vector.tensor_copy(dst[ec][:, isl], p)

                def emit_v(xa_t, ic, it, pool=None, ptag=None):
                    lc = 4 * ic + it
                    p = (pool or ps_pw).tile(
                        [128, 512], F32, tag=ptag or "pw", name="pv"
                    )
                    pv = p[:, 0:E]
                    for dc in range(NDC):
                        nc.tensor.matmul(
                            pv,
                            xa_t[:, dc, it * 128 : (it + 1) * 128],
                            wvr[:, dc, :],
                            start=(dc == 0),
                            stop=(dc == NDC - 1),
                        )
                    with nc.allow_low_precision(reason="bf16 V"):
                        for ec in range(2):
                            nc.vector.tensor_copy(
                                vt[ec][:, lc, :, 0:64],
                                pv[:, ec * 128 : (ec + 1) * 128].rearrange(
                                    "p (h v) -> p h v", h=2
                                ),
                            )

                def emit_u(xa_t, ic, it, pool=None, ptag=None):
                    lc = 4 * ic + it
                    p = (pool or ps_pw).tile(
                        [128, 512], F32, tag=ptag or "pw", name="pu"
                    )
                    pu = p[:, 0:E]
                    for dc in range(NDC):
                        nc.tensor.matmul(
                            pu,
                            xa_t[:, dc, it * 128 : (it + 1) * 128],
                            wur[:, dc, :],
                            start=(dc == 0),
                            stop=(dc == NDC - 1),
                        )
                    with nc.allow_low_precision(reason="bf16 U"):
                        nc.vector.tensor_copy(ut[:, lc, :], pu)

                ostate = {}

                def emit_wo(ic, it, fc, pool=None, ptag=None, copier=None):
                    lc = 4 * ic + it
                    if fc == 0:
                        ostate[lc] = ostg.tile([128, 1024], BF16, tag="o", name="ostg")
                    p = (pool or ps_pw).tile(
                        [128, 512], F32, tag=ptag or "pw", name="pwo"
                    )
                    for ec in range(2):
                        nc.tensor.matmul(
                            p,
                            gt[(ec, lc)],
                            wor[:, ec, fc * 512 : (fc + 1) * 512],
                            start=(ec == 0),
                            stop=(ec == 1),
                        )
                    o = ostate[lc]
                    cp = copier or nc.vector
                    with nc.allow_low_precision(reason="bf16 output"):
                        if cp is nc.scalar:
                            cp.copy(o[:, fc * 512 : (fc + 1) * 512], p)
                        else:
                            cp.tensor_copy(o[:, fc * 512 : (fc + 1) * 512], p)
                    if fc == 1:
                        nc.sync.dma_start(
                            out=out[lc * 128 : (lc + 1) * 128, :], in_=o
                        )

                def wo_filler(ic):
                    for it in range(4):
                        for fc in range(2):
                            yield 1024, (lambda ic=ic, it=it, fc=fc: emit_wo(ic, it, fc))

                def qu_filler(xa_t, ic, pool=None, ptag=None):
                    for ec in range(2):
                        yield 4096, (lambda ec=ec: emit_kq(
                            xa_t, ic, ec, wqr, qt, pool=pool, ptag=ptag
                        ))
                    for it in range(4):
                        yield 2048, (lambda it=it: emit_u(
                            xa_t, ic, it, pool=pool, ptag=ptag
                        ))

                # ---------- upfront: K/V for blocks 0-1; later blocks are
                # interleaved into ic=0's attention as filler work ----------
                nkb = -(-njc // 4)  # 512-blocks of keys needed
                half = NDC // 2
                xs = {}
                x0 = xap.tile([128, NDC, 512], F32R, tag="x", name="xa")
                xs[0] = x0
                nc.sync.dma_start(
                    out=x0[:, 0:half, :],
                    in_=xt[0 : half * 128, 0:512].rearrange(
                        "(dc p) i -> p dc i", p=128
                    ),
                )
                preload_early()
                nc.sync.dma_start(
                    out=x0[:, half:NDC, :],
                    in_=xt[half * 128 : D, 0:512].rearrange(
                        "(dc p) i -> p dc i", p=128
                    ),
                )
                preload_early2()
                nkb_up = min(nkb, 2)
                for ic in range(nkb_up):
                    if ic + 1 < NIC:
                        xs[ic + 1] = load_x(ic + 1)
                    if ic == 0:
                        preload_mid()
                    if ic == 1 or nkb_up == 1:
                        preload_late()
                    for ec in range(2):
                        emit_kq(xs[ic], ic, ec, wkr, kt, pool=ps_s, ptag="s")
                    for it in range(4):
                        if 4 * ic + it < njc:
                            emit_v(xs[ic], ic, it, pool=ps_s, ptag="s")
                for ic in range(nkb_up, NIC):
                    if ic + 1 < NIC:
                        xs[ic + 1] = load_x(ic + 1)

                def kv_filler(ic):
                    for ec in range(2):
                        yield 4096, (lambda ec=ec: emit_kq(xs[ic], ic, ec, wkr, kt))
                    for it in range(4):
                        if 4 * ic + it < njc:
                            yield 2048, (lambda it=it: emit_v(xs[ic], ic, it))

                # Q/U for ic=0 (emitted before attention)
                for _, f in qu_filler(xs[0], 0, pool=ps_s, ptag="s"):
                    f()

                def gate_ic(ic, avs, fuse_wo=False):
                    avsr = avs.rearrange("p (s c) -> p s c", c=65)
                    rec = rpool.tile([128, 16], F32, tag="rec", name="rec")
                    with nc.allow_low_precision(reason="softmax recip"):
                        nc.vector.reciprocal(rec, avsr[:, :, 64])
                    for it in range(4):
                        lc = 4 * ic + it
                        for ec in range(2):
                            g = gpool.tile([128, 128], BF16, tag="g", name="g")
                            with nc.allow_low_precision(reason="bf16 gating"):
                                for h in range(2):
                                    grp = 2 * ec + h
                                    k = it * 4 + grp
                                    nc.vector.scalar_tensor_tensor(
                                        g[:, h * 64 : (h + 1) * 64],
                                        avsr[:, k, 0:64],
                                        rec[:, k : k + 1],
                                        ut[:, lc, grp * 64 : (grp + 1) * 64],
                                        MULT,
                                        MULT,
                                    )
                            pt = (ps_s if fuse_wo else ps_pw).tile(
                                [128, 128], BF16,
                                tag="s" if fuse_wo else "pw", name="pt"
                            )
                            nc.tensor.transpose(pt, g, idt)
                            with nc.allow_low_precision(reason="bf16 gt"):
                                if fuse_wo:
                                    nc.scalar.copy(gt[(ec, lc)], pt)
                                else:
                                    nc.vector.tensor_copy(gt[(ec, lc)], pt)
                        if fuse_wo:
                            for fc in range(2):
                                emit_wo(
                                    ic, it, fc, pool=ps_s, ptag="s",
                                    copier=nc.scalar,
                                )

                # ---------- main loop ----------
                pend_gate = None
                for ic in range(NIC):
                    fillers = []
                    if ic == 0:
                        for b in range(nkb_up, nkb):
                            fillers.extend(kv_filler(b))
                    if ic > 0:
                        fillers.extend(wo_filler(ic - 1))
                    if ic + 1 < NIC:
                        fillers.extend(qu_filler(xs[ic + 1], ic + 1))
                    fillers.reverse()  # pop() from the front of the schedule

                    av = ps_av.tile([128, 1536], F32, tag="av", name="av")
                    av_banks = set(range(3))  # no start=True: memset below
                    nc.vector.memset(av, 0.0)
                    if pend_gate is not None:
                        gate_ic(*pend_gate)
                        pend_gate = None

                    isl = slice(ic * 512, (ic + 1) * 512)
                    wtot = sum(w for w, _ in fillers) or 1
                    npair = 2 * njc
                    pair = 0
                    wdone = 0
                    pend_av = []

                    def emit_av(e, jc, ec):
                        for h in range(2):
                            grp = 2 * ec + h
                            for it in range(4):
                                k = it * 4 + grp
                                bank = k // 7
                                st = bank not in av_banks
                                av_banks.add(bank)
                                nc.tensor.matmul(
                                    av[:, _slot(k) : _slot(k) + 65],
                                    e[:, h, it, :],
                                    vt[ec][:, jc, h, :],
                                    start=st,
                                    stop=False,
                                    skip_group_check=True,
                                )

                    for jc in range(njc):
                        jsl = slice(jc * 128, (jc + 1) * 128)
                        for ec in range(2):
                            stile = ps_s.tile([128, 1024], F32, tag="s", name="stile")
                            for h in range(2):
                                nc.tensor.matmul(
                                    stile[:, h * 512 : (h + 1) * 512],
                                    kt[ec][h * 64 : (h + 1) * 64, jsl],
                                    qt[ec][h * 64 : (h + 1) * 64, isl],
                                    start=True,
                                    stop=True,
                                )
                            # AV of an earlier pair runs while ACT does this
                            # pair's exp (2-deep software pipeline)
                            if len(pend_av) >= 6:
                                emit_av(*pend_av.pop(0))
                            # paced filler work (Q/U of ic+1, W_o of ic-1)
                            pair += 1
                            while fillers and wdone * npair < pair * wtot:
                                w, f = fillers.pop()
                                wdone += w
                                f()
                            e = epool.tile([128, 2, 4, 128], BF16, tag="e", name="e")
                            sv = stile.rearrange("p (h i) -> p h i", h=2)
                            if jc // 4 == ic:
                                # diagonal pair: columns below the diagonal
                                # subtile see the prompt-only (ab) bias; the
                                # diagonal subtile needs the 2D mask; columns
                                # above are causally free (bl bias)
                                t0 = jc % 4
                                w0, w1 = t0 * 128, (t0 + 1) * 128
                                nc.vector.tensor_add(
                                    sv[:, :, w0:w1],
                                    sv[:, :, w0:w1],
                                    dmt[:, jc, :].unsqueeze(1).broadcast_to(
                                        [128, 2, 128]
                                    ),
                                )
                                with nc.allow_low_precision(reason="bf16 softmax"):
                                    if t0 > 0:
                                        nc.scalar.activation(
                                            e[:, :, 0:t0, :],
                                            sv[:, :, 0:w0].rearrange(
                                                "p h (it i) -> p h it i", i=128
                                            ),
                                            EXP,
                                            bias=bab[:, jc : jc + 1],
                                            scale=1.0,
                                        )
                                    nc.scalar.activation(
                                        e[:, :, t0:4, :],
                                        sv[:, :, w0:512].rearrange(
                                            "p h (it i) -> p h it i", i=128
                                        ),
                                        EXP,
                                        bias=bbl[:, jc : jc + 1],
                                        scale=1.0,
                                    )
                            else:
                                bias = bab if jc // 4 > ic else bbl
                                with nc.allow_low_precision(reason="bf16 softmax"):
                                    nc.scalar.activation(
                                        e.rearrange("p h it i -> p (h it i)"),
                                        stile,
                                        EXP,
                                        bias=bias[:, jc : jc + 1],
                                        scale=1.0,
                                    )
                            pend_av.append((e, jc, ec))
                    for pa in pend_av:
                        emit_av(*pa)
                    while fillers:
                        fillers.pop()[1]()
                    # stage av to SBUF (bf16) so gating doesn't block next memset
                    avs = avsp.tile([128, 16 * 65], BF16, tag="avs", name="avs")
                    with nc.allow_low_precision(reason="bf16 av staging"):
                        for r in range(3):
                            cnt = min(7, 16 - 7 * r)
                            nc.vector.tensor_copy(
                                avs[:, 7 * r * 65 : (7 * r + cnt) * 65],
                                av[:, r * 512 : r * 512 + cnt * 65],
                            )
                    pend_gate = (ic, avs)

                gate_ic(*pend_gate, fuse_wo=True)

    nc.compile()
    return nc


def _host_inputs(njc, x, token_types, seq_lens, W_q, W_k, W_v, W_u, W_o):
    x = np.asarray(x, dtype=np.float32)
    token_types = np.asarray(token_types)
    seq_lens = np.asarray(seq_lens)
    W_q = np.asarray(W_q, dtype=np.float32)
    W_k = np.asarray(W_k, dtype=np.float32)
    W_v = np.asarray(W_v, dtype=np.float32)
    W_u = np.asarray(W_u, dtype=np.float32)
    W_o = np.asarray(W_o, dtype=np.float32)

    per_batch = []
    jr = np.arange(L)
    for b in range(B):
        xtb = np.ascontiguousarray(x[b].T)
        prompt = token_types[b] < 3
        valid = jr < int(seq_lens[b])
        ab = np.where(prompt & valid, 0.0, NEG).astype(np.float32)
        bl = np.where(valid, 0.0, NEG).astype(np.float32)
        biasab = np.ascontiguousarray(ab[: njc * 128].reshape(njc, 128).T)
        biasbl = np.ascontiguousarray(bl[: njc * 128].reshape(njc, 128).T)
        # diagonal-tile 2D masks, on top of the bl bias: NEG only where a
        # valid item key sits above the diagonal
        dmk = np.empty((njc, 128, 128), np.float32)
        for jc in range(njc):
            j = jr[jc * 128 : (jc + 1) * 128]
            i = np.arange(jc * 128, (jc + 1) * 128)  # diagonal subtile only
            allowed = valid[j][:, None] & (
                prompt[j][:, None] | (j[:, None] <= i[None, :])
            )
            full = np.where(allowed, 0.0, NEG).astype(np.float32)
            dmk[jc] = full - bl[j][:, None]
        per_batch.append((xtb, biasab, biasbl, dmk))

    in_maps = []
    for c in range(8):
        b, gi = c // 4, c % 4
        e0 = E * gi
        xtb, biasab, biasbl, dmk = per_batch[b]
        in_maps.append(
            {
                "xt": xtb,
                "wq": np.ascontiguousarray((W_q[e0 : e0 + E] / 8.0).T),
                "wk": np.ascontiguousarray(W_k[e0 : e0 + E].T),
                "wv": np.ascontiguousarray(W_v[e0 : e0 + E].T),
                "wu": np.ascontiguousarray(W_u[e0 : e0 + E].T),
                "wo": np.ascontiguousarray(W_o[:, e0 : e0 + E].T).astype(
                    ml_dtypes.bfloat16
                ),
                "ident": np.eye(128, dtype=np.float32).astype(ml_dtypes.bfloat16),
                "biasab": biasab,
                "biasbl": biasbl,
                "dmask": dmk.astype(ml_dtypes.bfloat16),
            }
        )
    return in_maps


def kernel(x, token_types, seq_lens, W_q, W_k, W_v, W_u, W_o, **_run_kwargs):
    seq_lens = np.asarray(seq_lens)
    njc = int(-(-int(seq_lens.max()) // 128))
    njc = max(1, min(njc, L // 128))
    if ("nc", njc) not in _cache:
        _cache[("nc", njc)] = _build(njc)
    nc = _cache[("nc", njc)]
    _cache["nc"] = nc
    in_maps = _host_inputs(njc, x, token_types, seq_lens, W_q, W_k, W_v, W_u, W_o)
    try:
        res = run_bass_kernel_spmd(nc, in_maps, list(range(8)), **_run_kwargs)
    except Exception as ex:  # transient NRT device wedge: retry once
        if "UNRECOVERABLE" not in str(ex) and "UNAVAILABLE" not in str(ex):
            raise
        res = run_bass_kernel_spmd(nc, in_maps, list(range(8)), **_run_kwargs)
    _cache["last_result"] = res
    full = np.zeros((B, L, D), np.float64)
    for c in range(8):
        full[c // 4] += res.results[c]["out"].astype(np.float64)
    return full.astype(np.float32)
